# revision 3
# baseline (speedup 1.0000x reference)
"""Trainium2 Bass kernel for CropConv: 3x3 same-padding conv (64->64 ch) on
[16, 64, 128, 128] fp32 input, with a static crop mask zeroing output rows/cols
[44:84).

Strategy (data-parallel over batch, 8 cores x 2 images each):
  - Host marshals x into a zero-padded row-major layout with row stride 129
    (131 padded rows: top pad, bottom pad, stream slack; the left zero column
    of each row doubles as the previous row's right pad), so every conv tap
    (kh, kw) of an output row-chunk is one contiguous rhs slice.
  - Per core, image 0 lives in SBUF partitions 0-63 (partition = in-channel),
    image 1 in partitions 64-127.
  - The conv is 9 PSUM-accumulated TensorE matmuls per output chunk:
    out[oc, pix] += W[kh,kw][ic, oc].T @ x[ic, shifted pix].  K = M = 64, so
    four matmuls run concurrently in the four 64x64 quadrants of the PE array
    (row-half = image, col-half = chunk pairing (c, c+22)).
  - Output path is bf16: PSUM -> SBUF stage copies convert to bf16, and the
    DRAM output layout IS the stage layout (partition p = rowhalf*64 + oc,
    free = (row, img, col) linear), so each store is a [128, n*256] fully
    contiguous DMA.  Stores fire every 2 pairs, overlapping compute.
  - No on-device masking: the host zeroes the static crop window and converts
    bf16 -> fp32 while unpacking the stage layout.
"""

import numpy as np

# ---- problem constants (hardcoded; kernel.py must be self-contained) ----
B, C, H, W = 16, 64, 128, 128
OC, KS = 64, 3
N_CORES = 8
IMGS = B // N_CORES  # 2 images per core

WP = W + 1            # padded row stride: 129
HP = H + 3            # padded rows in the x buffer: 131
XLEN = HP * WP        # 16899 bf16 per partition

RPC = 3               # output rows per chunk
NCH = (H + RPC - 1) // RPC          # 43 chunks per image (last has 2 rows)
NPAIR = 21            # chunk pairs (c, c+22); chunk 21 is the leftover
CHN = RPC * WP        # matmul free dim per full chunk: 387

# stage / output layout: [128, 66*2*128] bf16.
#   partition p = s*64 + oc, s = row half (0: y rows 0..65, 1: y rows 66..127)
#   free idx    = (r*2 + b)*128 + w   (r = row within half, b = image)
# upper half only uses r in [0, 62); its tail is never written or stored.
ROWH = 66             # rows in the lower half (upper half has 62)
RB = 2 * W            # free stride per stage row: 256
STLEN = ROWH * RB     # 16896

_CACHE = {}


def _build_module():
    import concourse.tile as tile
    from concourse import bacc, mybir

    f32 = mybir.dt.float32
    bf16 = mybir.dt.bfloat16

    nc = bacc.Bacc("TRN2", target_bir_lowering=False, debug=False,
                   num_devices=N_CORES)

    x_ap = nc.dram_tensor("xin", [IMGS, C, XLEN], bf16,
                          kind="ExternalInput").ap()
    w_ap = nc.dram_tensor("wt", [C, KS * KS, OC], bf16,
                          kind="ExternalInput").ap()
    y_ap = nc.dram_tensor("yout", [128, STLEN], bf16,
                          kind="ExternalOutput").ap()

    x_bc = x_ap.rearrange("b c l -> (b c) l")  # [128, XLEN]

    with tile.TileContext(nc) as tc:
        with tc.tile_pool(name="big", bufs=1) as big, \
             tc.tile_pool(name="psum", bufs=8, space="PSUM") as pp:

            x_sb = big.tile([128, XLEN], bf16, tag="xbuf")
            stage = big.tile([128, STLEN], bf16, tag="stage")
            w_sb = big.tile([128, KS * KS * OC], bf16, tag="wbuf")

            st = stage.rearrange("p (r b w) -> p r b w", b=IMGS, w=W)

            # weights first (tiny), replicated into both partition halves
            w_flat = w_ap.rearrange("i t o -> i (t o)")
            nc.gpsimd.dma_start(out=w_sb[0:64, :], in_=w_flat)
            nc.gpsimd.dma_start(out=w_sb[64:128, :], in_=w_flat)

            # x loads: fine-grained contiguous padded-row segments, ordered to
            # match the two consumption fronts (lower rows 0.., upper 66..)
            segs = [(0, 17), (66, 83), (17, 33), (83, 99),
                    (33, 49), (99, 115), (49, 66), (115, 131)]
            for i, (a, b_) in enumerate(segs):
                eng = (nc.gpsimd, nc.sync)[min(i, 1)]
                eng.dma_start(out=x_sb[:, a * WP:b_ * WP],
                              in_=x_bc[:, a * WP:b_ * WP])

            def lhsT(half, t):
                return w_sb[half * 64:(half + 1) * 64, t * OC:(t + 1) * OC]

            def rhs(half, c, kh, kw, n):
                off = (RPC * c + kh) * WP + kw
                return x_sb[half * 64:(half + 1) * 64, off:off + n]

            def chunk_n(c):
                return 2 * WP if c == NCH - 1 else CHN  # 258 for chunk 42

            TAPS = [(kh, kw) for kh in range(KS) for kw in range(KS)]

            def evict(psum_t, img, c, upper_nr):
                p3 = psum_t[:, 0:CHN].rearrange("p (h w) -> p h w", w=WP)
                r0 = RPC * (c if c < 22 else c - 22)
                nc.any.tensor_copy(st[0:64, r0:r0 + RPC, img, :],
                                   p3[0:64, 0:RPC, 0:W])
                nc.any.tensor_copy(st[64:128, r0:r0 + upper_nr, img, :],
                                   p3[64:128, 0:upper_nr, 0:W])

            for c in range(NPAIR):
                c2 = c + 22
                n2 = chunk_n(c2)
                pa = pp.tile([128, 512], f32, tag="ps")
                pb = pp.tile([128, 512], f32, tag="ps")
                for t, (kh, kw) in enumerate(TAPS):
                    sta, sp = (t == 0), (t == len(TAPS) - 1)
                    # img0 chunk c -> A[0:64];  img0 chunk c+22 -> A[64:128]
                    nc.tensor.matmul(pa[0:64, 0:CHN], lhsT(0, t),
                                     rhs(0, c, kh, kw, CHN), start=sta,
                                     stop=sp, skip_group_check=True)
                    nc.tensor.matmul(pa[64:128, 0:n2], lhsT(0, t),
                                     rhs(0, c2, kh, kw, n2), start=sta,
                                     stop=sp, skip_group_check=True)
                    # img1 chunk c -> B[0:64];  img1 chunk c+22 -> B[64:128]
                    nc.tensor.matmul(pb[0:64, 0:CHN], lhsT(1, t),
                                     rhs(1, c, kh, kw, CHN), start=sta,
                                     stop=sp, skip_group_check=True)
                    nc.tensor.matmul(pb[64:128, 0:n2], lhsT(1, t),
                                     rhs(1, c2, kh, kw, n2), start=sta,
                                     stop=sp, skip_group_check=True)

                nr2 = n2 // WP
                evict(pa, 0, c, nr2)
                evict(pb, 1, c, nr2)

                # store granule g after pair 2g+1: stage rows [6g, 6g+6) of
                # both halves -> one fully-contiguous [128, 1536] DMA
                if c % 2 == 1:
                    g = c // 2
                    o0, o1 = 6 * g * RB, (6 * g + 6) * RB
                    nc.scalar.dma_start(out=y_ap[:, o0:o1],
                                        in_=stage[:, o0:o1])

            # leftover chunk 21 (lower rows 63-66), both images
            pc_ = pp.tile([128, 512], f32, tag="ps")
            pd_ = pp.tile([128, 512], f32, tag="ps")
            for t, (kh, kw) in enumerate(TAPS):
                sta, sp = (t == 0), (t == len(TAPS) - 1)
                nc.tensor.matmul(pc_[0:64, 0:CHN], lhsT(0, t),
                                 rhs(0, 21, kh, kw, CHN), start=sta, stop=sp,
                                 skip_group_check=True)
                nc.tensor.matmul(pd_[0:64, 0:CHN], lhsT(1, t),
                                 rhs(1, 21, kh, kw, CHN), start=sta, stop=sp,
                                 skip_group_check=True)
            pc3 = pc_[:, 0:CHN].rearrange("p (h w) -> p h w", w=WP)
            pd3 = pd_[:, 0:CHN].rearrange("p (h w) -> p h w", w=WP)
            nc.any.tensor_copy(st[0:64, 63:66, 0, :], pc3[0:64, 0:3, 0:W])
            nc.any.tensor_copy(st[0:64, 63:66, 1, :], pd3[0:64, 0:3, 0:W])

            # tail stores: lower rows 60..66, upper rows 60..62
            nc.scalar.dma_start(out=y_ap[0:64, 60 * RB:66 * RB],
                                in_=stage[0:64, 60 * RB:66 * RB])
            nc.scalar.dma_start(out=y_ap[64:128, 60 * RB:62 * RB],
                                in_=stage[64:128, 60 * RB:62 * RB])

    nc.compile()
    return nc


def _get_module():
    if "nc" not in _CACHE:
        _CACHE["nc"] = _build_module()
    return _CACHE["nc"]


def _make_in_maps(x, weight):
    x = np.asarray(x, dtype=np.float32)
    weight = np.asarray(weight, dtype=np.float32)
    # host marshaling: pad x into the row-major stride-129 layout
    xp = np.zeros((B, C, HP, WP), dtype=np.float32)
    xp[:, :, 1:H + 1, 1:W + 1] = x
    xp = xp.reshape(B, C, XLEN)
    import ml_dtypes
    xp = xp.astype(ml_dtypes.bfloat16)
    # weight [oc, ic, kh, kw] -> [ic, (kh kw), oc]
    wt = np.ascontiguousarray(
        weight.transpose(1, 2, 3, 0).reshape(C, KS * KS, OC)
    ).astype(ml_dtypes.bfloat16)
    return [
        {"xin": np.ascontiguousarray(xp[k * IMGS:(k + 1) * IMGS]), "wt": wt}
        for k in range(N_CORES)
    ]


def kernel(x, weight):
    from concourse.bass_utils import run_bass_kernel_spmd

    nc = _get_module()
    in_maps = _make_in_maps(x, weight)
    res = run_bass_kernel_spmd(nc, in_maps, list(range(N_CORES)))
    out = np.empty((B, OC, H, W), dtype=np.float32)
    for k in range(N_CORES):
        a = np.asarray(res.results[k]["yout"]).reshape(128, ROWH, IMGS, W)
        blk = out[k * IMGS:(k + 1) * IMGS]
        # [oc, r, b, w] -> [b, oc, r, w]
        blk[:, :, 0:ROWH] = a[0:64].transpose(2, 0, 1, 3).astype(np.float32)
        blk[:, :, ROWH:H] = a[64:128, 0:H - ROWH].transpose(
            2, 0, 1, 3).astype(np.float32)
    # static crop mask: host zeroes rows/cols [44:84)
    out[:, :, 44:84, 44:84] = 0.0
    return out


# revision 4
# speedup vs baseline: 1.1102x; 1.1102x over previous
"""Trainium2 Bass kernel for CropConv: 3x3 same-padding conv (64->64 ch) on
[16, 64, 128, 128] fp32 input, with a static crop mask zeroing output rows/cols
[44:84).

Strategy (data-parallel over batch, 8 cores x 2 images each):
  - Host marshals x into a zero-padded row-major layout with row stride 129
    (131 padded rows: top pad, bottom pad, stream slack; the left zero column
    of each row doubles as the previous row's right pad), so every conv tap
    (kh, kw) of an output row-chunk is one contiguous rhs slice.
  - Per core, image 0 lives in SBUF partitions 0-63 (partition = in-channel),
    image 1 in partitions 64-127.
  - The conv is 9 PSUM-accumulated TensorE matmuls per output chunk:
    out[oc, pix] += W[kh,kw][ic, oc].T @ x[ic, shifted pix].  K = M = 64, so
    four matmuls run concurrently in the four 64x64 quadrants of the PE array
    (row-half = image, col-half = chunk pairing (c, c+22)).
  - Output path is bf16: PSUM -> SBUF stage copies convert to bf16, and the
    DRAM output layout IS the stage layout (partition p = rowhalf*64 + oc,
    free = (row, img, col) linear), so each store is a [128, n*256] fully
    contiguous DMA.  Stores fire every 2 pairs, overlapping compute.
  - Loads are issued on one queue in consumption-priority order (weights,
    then small lead segments for both row fronts) so compute starts early.
  - No on-device masking: the host zeroes the static crop window and converts
    bf16 -> fp32 while unpacking the stage layout.
"""

import numpy as np

# ---- problem constants (hardcoded; kernel.py must be self-contained) ----
B, C, H, W = 16, 64, 128, 128
OC, KS = 64, 3
N_CORES = 8
IMGS = B // N_CORES  # 2 images per core

WP = W + 1            # padded row stride: 129
HP = H + 3            # padded rows in the x buffer: 131
XLEN = HP * WP        # 16899 bf16 per partition

RPC = 3               # output rows per chunk
NCH = (H + RPC - 1) // RPC          # 43 chunks per image (last has 2 rows)
NPAIR = 21            # chunk pairs (c, c+22); chunk 21 is the leftover
CHN = RPC * WP        # matmul free dim per full chunk: 387

# stage / output layout: [128, 66*2*128] bf16.
#   partition p = s*64 + oc, s = row half (0: y rows 0..65, 1: y rows 66..127)
#   free idx    = (r*2 + b)*128 + w   (r = row within half, b = image)
# upper half only uses r in [0, 62); its tail is never written or stored.
ROWH = 66             # rows in the lower half (upper half has 62)
RB = 2 * W            # free stride per stage row: 256
STLEN = ROWH * RB     # 16896

_CACHE = {}


def _build_module():
    import concourse.tile as tile
    from concourse import bacc, mybir

    f32 = mybir.dt.float32
    bf16 = mybir.dt.bfloat16

    nc = bacc.Bacc("TRN2", target_bir_lowering=False, debug=False,
                   num_devices=N_CORES)

    x_ap = nc.dram_tensor("xin", [IMGS, C, XLEN], bf16,
                          kind="ExternalInput").ap()
    w_ap = nc.dram_tensor("wt", [C, KS * KS, OC], bf16,
                          kind="ExternalInput").ap()
    y_ap = nc.dram_tensor("yout", [128, STLEN], bf16,
                          kind="ExternalOutput").ap()

    x_bc = x_ap.rearrange("b c l -> (b c) l")  # [128, XLEN]

    with tile.TileContext(nc) as tc:
        with tc.tile_pool(name="big", bufs=1) as big, \
             tc.tile_pool(name="psum", bufs=8, space="PSUM") as pp:

            x_sb = big.tile([128, XLEN], bf16, tag="xbuf")
            stage = big.tile([128, STLEN], bf16, tag="stage")
            w_sb = big.tile([128, KS * KS * OC], bf16, tag="wbuf")

            st = stage.rearrange("p (r b w) -> p r b w", b=IMGS, w=W)

            # single prioritized load queue: weights first (tiny), then small
            # lead segments of both consumption fronts, then the rest
            w_flat = w_ap.rearrange("i t o -> i (t o)")
            nc.sync.dma_start(out=w_sb[0:64, :], in_=w_flat)
            nc.sync.dma_start(out=w_sb[64:128, :], in_=w_flat)

            segs = [(0, 8), (66, 74), (8, 24), (74, 90),
                    (24, 44), (90, 110), (44, 66), (110, 131)]
            for (a, b_) in segs:
                nc.sync.dma_start(out=x_sb[:, a * WP:b_ * WP],
                                  in_=x_bc[:, a * WP:b_ * WP])

            def lhsT(half, t):
                return w_sb[half * 64:(half + 1) * 64, t * OC:(t + 1) * OC]

            def rhs(half, c, kh, kw, n):
                off = (RPC * c + kh) * WP + kw
                return x_sb[half * 64:(half + 1) * 64, off:off + n]

            TAPS = [(kh, kw) for kh in range(KS) for kw in range(KS)]

            def pair_unit(c):
                """Pairs (c, c+22): 4 PE quadrants, 2 PSUM banks, then evict
                into the stage (merged 128-partition copies when possible)."""
                c2 = c + 22
                n2 = 2 * WP if c2 == NCH - 1 else CHN  # 258 for chunk 42
                pa = pp.tile([128, 512], f32, tag="ps")
                pb = pp.tile([128, 512], f32, tag="ps")
                for t, (kh, kw) in enumerate(TAPS):
                    sta, sp = (t == 0), (t == len(TAPS) - 1)
                    nc.tensor.matmul(pa[0:64, 0:CHN], lhsT(0, t),
                                     rhs(0, c, kh, kw, CHN), start=sta,
                                     stop=sp, skip_group_check=True)
                    nc.tensor.matmul(pa[64:128, 0:n2], lhsT(0, t),
                                     rhs(0, c2, kh, kw, n2), start=sta,
                                     stop=sp, skip_group_check=True)
                    nc.tensor.matmul(pb[0:64, 0:CHN], lhsT(1, t),
                                     rhs(1, c, kh, kw, CHN), start=sta,
                                     stop=sp, skip_group_check=True)
                    nc.tensor.matmul(pb[64:128, 0:n2], lhsT(1, t),
                                     rhs(1, c2, kh, kw, n2), start=sta,
                                     stop=sp, skip_group_check=True)
                r0 = RPC * c
                for img, pt in ((0, pa), (1, pb)):
                    p3 = pt[:, 0:CHN].rearrange("p (h w) -> p h w", w=WP)
                    if n2 == CHN:
                        # both halves share row range/offsets: one 128-p copy
                        nc.any.tensor_copy(st[:, r0:r0 + RPC, img, :],
                                           p3[:, 0:RPC, 0:W])
                    else:
                        nr2 = n2 // WP
                        nc.any.tensor_copy(st[0:64, r0:r0 + RPC, img, :],
                                           p3[0:64, 0:RPC, 0:W])
                        nc.any.tensor_copy(st[64:128, r0:r0 + nr2, img, :],
                                           p3[64:128, 0:nr2, 0:W])

            def leftover_unit():
                """Chunk 21 (lower rows 63-66), both images, 2 PE quadrants."""
                pc_ = pp.tile([128, 512], f32, tag="ps")
                pd_ = pp.tile([128, 512], f32, tag="ps")
                for t, (kh, kw) in enumerate(TAPS):
                    sta, sp = (t == 0), (t == len(TAPS) - 1)
                    nc.tensor.matmul(pc_[0:64, 0:CHN], lhsT(0, t),
                                     rhs(0, 21, kh, kw, CHN), start=sta,
                                     stop=sp, skip_group_check=True)
                    nc.tensor.matmul(pd_[0:64, 0:CHN], lhsT(1, t),
                                     rhs(1, 21, kh, kw, CHN), start=sta,
                                     stop=sp, skip_group_check=True)
                for img, pt in ((0, pc_), (1, pd_)):
                    p3 = pt[:, 0:CHN].rearrange("p (h w) -> p h w", w=WP)
                    nc.any.tensor_copy(st[0:64, 63:66, img, :],
                                       p3[0:64, 0:3, 0:W])

            for c in range(NPAIR):
                pair_unit(c)
                if c == 9:
                    # leftover mid-schedule so its evict/store isn't the tail
                    leftover_unit()
                if c % 2 == 1:
                    # store granule: stage rows [6g, 6g+6) of both halves as
                    # one fully-contiguous [128, 1536] DMA
                    g = c // 2
                    o0, o1 = 6 * g * RB, (6 * g + 6) * RB
                    nc.scalar.dma_start(out=y_ap[:, o0:o1],
                                        in_=stage[:, o0:o1])

            # tail stores: lower rows 60..66, upper rows 60..62
            nc.scalar.dma_start(out=y_ap[0:64, 60 * RB:66 * RB],
                                in_=stage[0:64, 60 * RB:66 * RB])
            nc.scalar.dma_start(out=y_ap[64:128, 60 * RB:62 * RB],
                                in_=stage[64:128, 60 * RB:62 * RB])

    nc.compile()
    return nc


def _get_module():
    if "nc" not in _CACHE:
        _CACHE["nc"] = _build_module()
    return _CACHE["nc"]


def _make_in_maps(x, weight):
    x = np.asarray(x, dtype=np.float32)
    weight = np.asarray(weight, dtype=np.float32)
    # host marshaling: pad x into the row-major stride-129 layout
    xp = np.zeros((B, C, HP, WP), dtype=np.float32)
    xp[:, :, 1:H + 1, 1:W + 1] = x
    xp = xp.reshape(B, C, XLEN)
    import ml_dtypes
    xp = xp.astype(ml_dtypes.bfloat16)
    # weight [oc, ic, kh, kw] -> [ic, (kh kw), oc]
    wt = np.ascontiguousarray(
        weight.transpose(1, 2, 3, 0).reshape(C, KS * KS, OC)
    ).astype(ml_dtypes.bfloat16)
    return [
        {"xin": np.ascontiguousarray(xp[k * IMGS:(k + 1) * IMGS]), "wt": wt}
        for k in range(N_CORES)
    ]


def kernel(x, weight):
    from concourse.bass_utils import run_bass_kernel_spmd

    nc = _get_module()
    in_maps = _make_in_maps(x, weight)
    res = run_bass_kernel_spmd(nc, in_maps, list(range(N_CORES)))
    out = np.empty((B, OC, H, W), dtype=np.float32)
    for k in range(N_CORES):
        a = np.asarray(res.results[k]["yout"]).reshape(128, ROWH, IMGS, W)
        blk = out[k * IMGS:(k + 1) * IMGS]
        # [oc, r, b, w] -> [b, oc, r, w]
        blk[:, :, 0:ROWH] = a[0:64].transpose(2, 0, 1, 3).astype(np.float32)
        blk[:, :, ROWH:H] = a[64:128, 0:H - ROWH].transpose(
            2, 0, 1, 3).astype(np.float32)
    # static crop mask: host zeroes rows/cols [44:84)
    out[:, :, 44:84, 44:84] = 0.0
    return out


# revision 5
# speedup vs baseline: 1.1222x; 1.0108x over previous
"""Trainium2 Bass kernel for CropConv: 3x3 same-padding conv (64->64 ch) on
[16, 64, 128, 128] fp32 input, with a static crop mask zeroing output rows/cols
[44:84).

Strategy (data-parallel over batch, 8 cores x 2 images each):
  - Host marshals x into a zero-padded row-major layout with row stride 129
    (131 padded rows; the left zero column of each row doubles as the previous
    row's right pad), bf16.
  - Per core, image 0 lives in SBUF partitions 0-63 (partition = in-channel),
    image 1 in partitions 64-127.
  - Output rows are processed in 4-row chunks (32 per image).  Each chunk is
    9 PSUM-accumulated TensorE matmuls (one per conv tap): free dim is a
    [4 rows x 128 cols] strided access pattern (512 elements = exactly one
    PSUM bank), skipping the pad column.  K = M = 64, so four matmuls run
    concurrently in the four 64x64 PE quadrants: row-half = image, col-half =
    chunk pairing (u, u+16) (= y row halves 0..64 / 64..128).
  - Output path is bf16: PSUM -> SBUF stage copies convert to bf16 (one
    128-partition copy per image per unit), and the DRAM output layout IS the
    stage layout (partition p = rowhalf*64 + oc, free = (row, img, col)
    linear), so each store is a [128, 2048] fully contiguous DMA.  Stores
    fire every 2 units, overlapping compute.
  - Loads are issued on one queue in consumption-priority order (weights,
    then small lead segments for both row fronts) so compute starts early.
  - No on-device masking: the host zeroes the static crop window and converts
    bf16 -> fp32 while unpacking the stage layout.
"""

import numpy as np

# ---- problem constants (hardcoded; kernel.py must be self-contained) ----
B, C, H, W = 16, 64, 128, 128
OC, KS = 64, 3
N_CORES = 8
IMGS = B // N_CORES  # 2 images per core

WP = W + 1            # padded row stride: 129
HP = H + 3            # padded rows in the x buffer: 131
XLEN = HP * WP        # 16899 bf16 per partition

RPC = 4               # output rows per chunk
NCHK = H // RPC       # 32 chunks per image
NUNIT = NCHK // 2     # 16 pair units (u, u+16)
FREE = RPC * W        # matmul free size: 512 (= 1 PSUM bank of fp32)

# stage / output layout: [128, 64*2*128] bf16.
#   partition p = s*64 + oc, s = row half (0: y rows 0..64, 1: y rows 64..128)
#   free idx    = (r*2 + b)*128 + w   (r = row within half, b = image)
ROWH = H // 2         # 64 rows per half
RB = 2 * W            # free stride per stage row: 256
STLEN = ROWH * RB     # 16384

_CACHE = {}


def _build_module():
    import concourse.tile as tile
    from concourse import bacc, mybir

    f32 = mybir.dt.float32
    bf16 = mybir.dt.bfloat16

    nc = bacc.Bacc("TRN2", target_bir_lowering=False, debug=False,
                   num_devices=N_CORES)

    x_ap = nc.dram_tensor("xin", [IMGS, C, XLEN], bf16,
                          kind="ExternalInput").ap()
    w_ap = nc.dram_tensor("wt", [C, KS * KS, OC], bf16,
                          kind="ExternalInput").ap()
    y_ap = nc.dram_tensor("yout", [128, STLEN], bf16,
                          kind="ExternalOutput").ap()

    x_bc = x_ap.rearrange("b c l -> (b c) l")  # [128, XLEN]

    with tile.TileContext(nc) as tc:
        with tc.tile_pool(name="big", bufs=1) as big, \
             tc.tile_pool(name="psum", bufs=8, space="PSUM") as pp:

            x_sb = big.tile([128, XLEN], bf16, tag="xbuf")
            stage = big.tile([128, STLEN], bf16, tag="stage")
            w_sb = big.tile([128, KS * KS * OC], bf16, tag="wbuf")

            st = stage.rearrange("p (r b w) -> p r b w", b=IMGS, w=W)
            # row views of x: x4 covers cols j = w+kw for kw in {0,1}; x4b is
            # shifted +2 so its row R covers j = w+2 (the kw=2 tap), where
            # col 129 of a row = the next row's zero left-pad (right-pad trick)
            x4 = x_sb.rearrange("p (r j) -> p r j", j=WP)          # [.,131,129]
            x4b = x_sb[:, 2:2 + 130 * WP].rearrange(
                "p (r j) -> p r j", j=WP)                          # [.,130,129]

            # single prioritized load queue: weights first (tiny), then small
            # lead segments of both consumption fronts, then the rest
            w_flat = w_ap.rearrange("i t o -> i (t o)")
            nc.sync.dma_start(out=w_sb[0:64, :], in_=w_flat)
            nc.sync.dma_start(out=w_sb[64:128, :], in_=w_flat)

            segs = [(0, 8), (64, 72), (8, 24), (72, 88),
                    (24, 44), (88, 108), (44, 64), (108, 131)]
            for (a, b_) in segs:
                nc.sync.dma_start(out=x_sb[:, a * WP:b_ * WP],
                                  in_=x_bc[:, a * WP:b_ * WP])

            def lhsT(half, t):
                return w_sb[half * 64:(half + 1) * 64, t * OC:(t + 1) * OC]

            def rhs(half, c, kh, kw):
                h0, h1 = half * 64, (half + 1) * 64
                R = RPC * c + kh
                if kw == 2:
                    return x4b[h0:h1, R:R + RPC, 0:W]
                return x4[h0:h1, R:R + RPC, kw:kw + W]

            TAPS = [(kh, kw) for kh in range(KS) for kw in range(KS)]

            for u in range(NUNIT):
                c2 = u + NCHK // 2
                pa = pp.tile([128, FREE], f32, tag="ps")
                pb = pp.tile([128, FREE], f32, tag="ps")
                for t, (kh, kw) in enumerate(TAPS):
                    sta, sp = (t == 0), (t == len(TAPS) - 1)
                    nc.tensor.matmul(pa[0:64, :], lhsT(0, t),
                                     rhs(0, u, kh, kw), start=sta,
                                     stop=sp, skip_group_check=True)
                    nc.tensor.matmul(pa[64:128, :], lhsT(0, t),
                                     rhs(0, c2, kh, kw), start=sta,
                                     stop=sp, skip_group_check=True)
                    nc.tensor.matmul(pb[0:64, :], lhsT(1, t),
                                     rhs(1, u, kh, kw), start=sta,
                                     stop=sp, skip_group_check=True)
                    nc.tensor.matmul(pb[64:128, :], lhsT(1, t),
                                     rhs(1, c2, kh, kw), start=sta,
                                     stop=sp, skip_group_check=True)
                r0 = RPC * u
                for img, pt in ((0, pa), (1, pb)):
                    pe = pt[:, :].rearrange("p (h w) -> p h w", w=W)
                    nc.any.tensor_copy(st[:, r0:r0 + RPC, img, :],
                                       pe[:, 0:RPC, :])

                if u % 2 == 1:
                    # store granule: stage rows [8g, 8g+8) of both halves as
                    # one fully-contiguous [128, 2048] DMA
                    g = u // 2
                    o0, o1 = 8 * g * RB, (8 * g + 8) * RB
                    nc.scalar.dma_start(out=y_ap[:, o0:o1],
                                        in_=stage[:, o0:o1])

    nc.compile()
    return nc


def _get_module():
    if "nc" not in _CACHE:
        _CACHE["nc"] = _build_module()
    return _CACHE["nc"]


def _make_in_maps(x, weight):
    x = np.asarray(x, dtype=np.float32)
    weight = np.asarray(weight, dtype=np.float32)
    # host marshaling: pad x into the row-major stride-129 layout
    xp = np.zeros((B, C, HP, WP), dtype=np.float32)
    xp[:, :, 1:H + 1, 1:W + 1] = x
    xp = xp.reshape(B, C, XLEN)
    import ml_dtypes
    xp = xp.astype(ml_dtypes.bfloat16)
    # weight [oc, ic, kh, kw] -> [ic, (kh kw), oc]
    wt = np.ascontiguousarray(
        weight.transpose(1, 2, 3, 0).reshape(C, KS * KS, OC)
    ).astype(ml_dtypes.bfloat16)
    return [
        {"xin": np.ascontiguousarray(xp[k * IMGS:(k + 1) * IMGS]), "wt": wt}
        for k in range(N_CORES)
    ]


def kernel(x, weight):
    from concourse.bass_utils import run_bass_kernel_spmd

    nc = _get_module()
    in_maps = _make_in_maps(x, weight)
    res = run_bass_kernel_spmd(nc, in_maps, list(range(N_CORES)))
    out = np.empty((B, OC, H, W), dtype=np.float32)
    for k in range(N_CORES):
        a = np.asarray(res.results[k]["yout"]).reshape(128, ROWH, IMGS, W)
        blk = out[k * IMGS:(k + 1) * IMGS]
        # [oc, r, b, w] -> [b, oc, r, w]
        blk[:, :, 0:ROWH] = a[0:64].transpose(2, 0, 1, 3).astype(np.float32)
        blk[:, :, ROWH:H] = a[64:128].transpose(2, 0, 1, 3).astype(np.float32)
    # static crop mask: host zeroes rows/cols [44:84)
    out[:, :, 44:84, 44:84] = 0.0
    return out


# revision 10
# speedup vs baseline: 1.1719x; 1.0443x over previous
"""Trainium2 Bass kernel for CropConv: 3x3 same-padding conv (64->64 ch) on
[16, 64, 128, 128] fp32 input, with a static crop mask zeroing output rows/cols
[44:84).

Strategy (data-parallel over batch, 8 cores x 2 images each):
  - Host marshals x into a zero-padded row-major layout with row stride 129
    (131 padded rows; the left zero column of each row doubles as the previous
    row's right pad), bf16.
  - Per core, image 0 lives in SBUF partitions 0-63 (partition = in-channel),
    image 1 in partitions 64-127.
  - Output rows are processed in 4-row chunks (32 per image).  Each chunk is
    9 PSUM-accumulated TensorE matmuls (one per conv tap): free dim is a
    [4 rows x 128 cols] strided access pattern (512 elements = exactly one
    PSUM bank), skipping the pad column.  K = M = 64, so four matmuls run
    concurrently in the four 64x64 PE quadrants: row-half = image, col-half =
    chunk pairing (u, u+16) (= y row halves 0..64 / 64..128).
  - Output path is bf16: PSUM -> SBUF stage copies convert to bf16 (one
    128-partition copy per image per unit), and the DRAM output layout IS the
    stage layout (partition p = rowhalf*64 + oc, free = (row, img, col)
    linear), so each store is a [128, 2048] fully contiguous DMA.  Stores
    fire every 2 units, overlapping compute.
  - Loads are issued on one queue in consumption-priority order (weights,
    then small lead segments for both row fronts) so compute starts early.
  - No on-device masking: the host zeroes the static crop window and converts
    bf16 -> fp32 while unpacking the stage layout.
"""

import numpy as np

# ---- problem constants (hardcoded; kernel.py must be self-contained) ----
B, C, H, W = 16, 64, 128, 128
OC, KS = 64, 3
N_CORES = 8
IMGS = B // N_CORES  # 2 images per core

WP = W + 1            # padded row stride: 129
HP = H + 3            # padded rows in the x buffer: 131
XLEN = HP * WP        # 16899 bf16 per partition

RPC = 4               # output rows per chunk
NCHK = H // RPC       # 32 chunks per image
NUNIT = NCHK // 2     # 16 pair units (u, u+16)
FREE = RPC * W        # matmul free size: 512 (= 1 PSUM bank of fp32)

# stage / output layout: [128, 64*2*128] bf16.
#   partition p = s*64 + oc, s = row half (0: y rows 0..64, 1: y rows 64..128)
#   free idx    = (r*2 + b)*128 + w   (r = row within half, b = image)
ROWH = H // 2         # 64 rows per half
RB = 2 * W            # free stride per stage row: 256
STLEN = ROWH * RB     # 16384

_CACHE = {}


def _build_module():
    import concourse.tile as tile
    from concourse import bacc, mybir

    f32 = mybir.dt.float32
    bf16 = mybir.dt.bfloat16

    nc = bacc.Bacc("TRN2", target_bir_lowering=False, debug=False,
                   num_devices=N_CORES)

    x_ap = nc.dram_tensor("xin", [IMGS, C, XLEN], bf16,
                          kind="ExternalInput").ap()
    w_ap = nc.dram_tensor("wt", [2 * C, KS * KS, OC], bf16,
                          kind="ExternalInput").ap()
    y_ap = nc.dram_tensor("yout", [128, STLEN], bf16,
                          kind="ExternalOutput").ap()

    x_bc = x_ap.rearrange("b c l -> (b c) l")  # [128, XLEN]

    with tile.TileContext(nc) as tc:
        with tc.tile_pool(name="big", bufs=1) as big, \
             tc.tile_pool(name="psum", bufs=8, space="PSUM") as pp:

            x_sb = big.tile([128, XLEN], bf16, tag="xbuf")
            stage = big.tile([128, STLEN], bf16, tag="stage")
            w_sb = big.tile([128, KS * KS * OC], bf16, tag="wbuf")

            st = stage.rearrange("p (r b w) -> p r b w", b=IMGS, w=W)
            # row views of x: x4 covers cols j = w+kw for kw in {0,1}; x4b is
            # shifted +2 so its row R covers j = w+2 (the kw=2 tap), where
            # col 129 of a row = the next row's zero left-pad (right-pad trick)
            x4 = x_sb.rearrange("p (r j) -> p r j", j=WP)          # [.,131,129]
            x4b = x_sb[:, 2:2 + 130 * WP].rearrange(
                "p (r j) -> p r j", j=WP)                          # [.,130,129]

            # prioritized loads: weights first (tiny, host-replicated to both
            # halves), then small lead segments of both consumption fronts
            # (upper lead on the otherwise-idle gpsimd queue so its trigger
            # doesn't serialize behind the sync queue), then the rest
            w_flat = w_ap.rearrange("i t o -> i (t o)")  # [128, 1152]
            nc.sync.dma_start(out=w_sb[:, :], in_=w_flat)
            nc.gpsimd.dma_start(out=x_sb[:, 64 * WP:71 * WP],
                                in_=x_bc[:, 64 * WP:71 * WP])

            segs = [(0, 7), (7, 24), (71, 88), (24, 44),
                    (88, 108), (44, 64), (108, 131)]
            for (a, b_) in segs:
                nc.sync.dma_start(out=x_sb[:, a * WP:b_ * WP],
                                  in_=x_bc[:, a * WP:b_ * WP])

            def lhsT(half, t):
                return w_sb[half * 64:(half + 1) * 64, t * OC:(t + 1) * OC]

            def rhs(half, c, kh, kw):
                h0, h1 = half * 64, (half + 1) * 64
                R = RPC * c + kh
                if kw == 2:
                    return x4b[h0:h1, R:R + RPC, 0:W]
                return x4[h0:h1, R:R + RPC, kw:kw + W]

            TAPS = [(kh, kw) for kh in range(KS) for kw in range(KS)]

            for u in range(NUNIT):
                c2 = u + NCHK // 2
                pa = pp.tile([128, FREE], f32, tag="ps")
                pb = pp.tile([128, FREE], f32, tag="ps")
                for t, (kh, kw) in enumerate(TAPS):
                    sta, sp = (t == 0), (t == len(TAPS) - 1)
                    nc.tensor.matmul(pa[0:64, :], lhsT(0, t),
                                     rhs(0, u, kh, kw), start=sta,
                                     stop=sp, skip_group_check=True)
                    nc.tensor.matmul(pa[64:128, :], lhsT(0, t),
                                     rhs(0, c2, kh, kw), start=sta,
                                     stop=sp, skip_group_check=True)
                    nc.tensor.matmul(pb[0:64, :], lhsT(1, t),
                                     rhs(1, u, kh, kw), start=sta,
                                     stop=sp, skip_group_check=True)
                    nc.tensor.matmul(pb[64:128, :], lhsT(1, t),
                                     rhs(1, c2, kh, kw), start=sta,
                                     stop=sp, skip_group_check=True)
                r0 = RPC * u
                for img, pt in ((0, pa), (1, pb)):
                    pe = pt[:, :].rearrange("p (h w) -> p h w", w=W)
                    nc.any.tensor_copy(st[:, r0:r0 + RPC, img, :],
                                       pe[:, 0:RPC, :])

                # stores: 8-row granules [8g, 8g+8) of both halves as one
                # fully-contiguous [128, 2048] DMA; finer 4-row stores at the
                # end to shrink the exposed tail
                bounds = None
                if u % 2 == 1 and u <= 11:
                    bounds = (8 * (u // 2), 8 * (u // 2) + 8)
                elif u == 13:
                    bounds = (48, 56)
                elif u >= 14:
                    bounds = (RPC * u, RPC * (u + 1))
                if bounds:
                    o0, o1 = bounds[0] * RB, bounds[1] * RB
                    nc.scalar.dma_start(out=y_ap[:, o0:o1],
                                        in_=stage[:, o0:o1])

    nc.compile()
    return nc


def _get_module():
    if "nc" not in _CACHE:
        _CACHE["nc"] = _build_module()
    return _CACHE["nc"]


def _make_in_maps(x, weight):
    x = np.asarray(x, dtype=np.float32)
    weight = np.asarray(weight, dtype=np.float32)
    # host marshaling: pad x into the row-major stride-129 layout
    xp = np.zeros((B, C, HP, WP), dtype=np.float32)
    xp[:, :, 1:H + 1, 1:W + 1] = x
    xp = xp.reshape(B, C, XLEN)
    import ml_dtypes
    xp = xp.astype(ml_dtypes.bfloat16)
    # weight [oc, ic, kh, kw] -> [ic, (kh kw), oc], replicated to both halves
    w1 = weight.transpose(1, 2, 3, 0).reshape(C, KS * KS, OC)
    wt = np.ascontiguousarray(
        np.concatenate([w1, w1], axis=0)).astype(ml_dtypes.bfloat16)
    return [
        {"xin": np.ascontiguousarray(xp[k * IMGS:(k + 1) * IMGS]), "wt": wt}
        for k in range(N_CORES)
    ]


def kernel(x, weight):
    from concourse.bass_utils import run_bass_kernel_spmd

    nc = _get_module()
    in_maps = _make_in_maps(x, weight)
    res = run_bass_kernel_spmd(nc, in_maps, list(range(N_CORES)))
    out = np.empty((B, OC, H, W), dtype=np.float32)
    for k in range(N_CORES):
        a = np.asarray(res.results[k]["yout"]).reshape(128, ROWH, IMGS, W)
        blk = out[k * IMGS:(k + 1) * IMGS]
        # [oc, r, b, w] -> [b, oc, r, w]
        blk[:, :, 0:ROWH] = a[0:64].transpose(2, 0, 1, 3).astype(np.float32)
        blk[:, :, ROWH:H] = a[64:128].transpose(2, 0, 1, 3).astype(np.float32)
    # static crop mask: host zeroes rows/cols [44:84)
    out[:, :, 44:84, 44:84] = 0.0
    return out
